# revision 30
# baseline (speedup 1.0000x reference)
"""Trainium2 8-core GCN kernel (2-layer GCNConv + linear head + softmax).

Strategy (node/row partitioning, dense normalized adjacency):
  - Host: build Ahat = D^-1/2 (A+I) D^-1/2 as a dense fp8-e4m3 matrix, padded
    from 10000 to 10240 nodes; core k owns node rows [k*1280, (k+1)*1280).
  - Device, per core k (all matmuls fp8-e4m3 DoubleRow, fp32 accumulate):
      y_k    = x^T Ahat^T[:,k]  (transposed SpMM, swept in column chunks)
      h1_k   = relu(W1^T y_k + b1)          (associativity: (A x) W1)
      t2_k   = (h1_k)^T @ W2                (h1T is directly the lhsT)
      t2     = AllGather(t2_k), one chunk per layer-1 sweep (2MB+2MB+1MB)
      h2T_k  = relu(t2^T Ahat^T[:,k] + b2)
      out_k  = softmax(h2T_k^T @ Wout + bout) ([1280, 16] f32)
  - Host: concatenate core outputs, trim padding to [10000, 16].

Schedule (what makes this fast; measured on HW):
  - The cores launch with a consistent ~40-55us stagger (constant per-chunk
    doorbell lag across the run -- an offset, not a clock-rate skew), and
    ncfw (the collective firmware, a SERIAL engine) adds a ~60us
    wake-from-idle before its first mesh; queued doorbells start in ~2us.
    Two small dummy AllGathers ring at ~15us so the wake burns off the
    critical path; each real 1MB chunk costs ~13-21us of mesh service
    (mostly slowest-peer waits inside).
  - Layer 1 runs four column sweeps -- 512:1024 (512-wide), 0:256,
    256:512, 1024:1280 (256-wide) -- each followed by its W1 GEMM,
    t2 GEMM, and 1MB AllGather chunk(s) (the first sweep ships two), so
    doorbells ring every ~20us from ~60us on and the 5.24MB wire runs as
    a continuously-fed conveyor, finishing ~188us.  The FIRST sweep must
    be 512-wide: its xpair+ATr co-load paces at the shared-SDMA ~220GB/s
    ceiling, and a narrow first sweep stretches 2x.  (Going narrower
    everywhere is strictly worse: ~46ns fixed cost per matmul, measured
    +36us tensor-busy at width-256 everywhere.)
  - Const loads go AFTER the whole xpair stream (a mid-stream burst shows
    up 1:1 as a tensor gap); each sweep's trailing GEMMs are emitted after
    the NEXT sweep's first j-pairs (alternating PSUM tag sets) so psum
    evacuation never bubbles the tensor queue.
  - Layer 2 runs as pass 1 = cols 0:1024 (all 8 PSUM banks: streamed
    0:512 + resident 512:1024) consuming j-pairs in chunk-arrival order,
    then a short pass 2 = cols 1024:1280 -- this puts only ~14us of
    pass-1 tail plus pass 2 (~21us) after the last chunk's data, vs ~50us
    for an even S/R split.  Heads for m0..7 overlap pass 2.
  - Layer-2 pass 1 reads BOTH adjacency halves from SBUF: cols 512:1024
    are resident from layer-1's first sweep, and cols 0:512 are refilled
    into the dead x tiles (same shape) on the idle sync queue -- streaming
    them JIT under the resident reads cost ~20us of SBUF port contention.
  - t2 reloads ride the scalar queue (drained by the time each mesh
    lands); on the sync queue they'd sit behind the 5.24MB refill and
    delay pass 1 by ~11us.  Their per-chunk AllGather gates match arrival
    order, so they cannot head-of-line-block each other.

The transposed SpMM (z^T = t^T A^T instead of z = A t) makes each layer's
activation land in [feature, node] layout, which is exactly the lhsT the
following GEMM needs -- no on-device transposes anywhere.  All matmuls use
perf_mode=DoubleRow (256 contraction rows per matmul): lhsT/rhs are
[128, 2, free] pair tiles, element [p, q] = contraction row q*128+p.
"""

import contextlib
import ctypes
import sys
import types

import ml_dtypes
import numpy as np

import concourse.bass as bass
import concourse.mybir as mybir
import concourse.tile as tile
from concourse.bass_utils import run_bass_kernel_spmd

BF16 = ml_dtypes.bfloat16
FP8 = ml_dtypes.float8_e4m3

N_CORES = 8
N_NODES = 10000
F_IN = 512
F_HID = 512
N_CLASSES = 16
NP = 10240            # padded node count (80 * 128)
R = NP // N_CORES     # 1280 rows per core
P = 128
NJ = NP // P          # 80 contraction chunks
NJP = NJ // 2         # 40 DoubleRow contraction pairs
NM = R // P           # 10 row tiles per core
NF = F_HID // P       # 4 feature tiles
NFP = NF // 2         # 2 feature pairs

# Layer-1 sweeps in execution order: (col offset, width, m-tiles).  The
# first sweep is 512-wide (its xpair+ATr co-load paces at the shared-SDMA
# ~220GB/s ceiling -- a narrow first sweep would stretch 2x); the rest are
# 256-wide so the AllGather conveyor gets doorbells every ~20us and each
# chunk's (arrival + remaining-pass-1-work) is flat across chunks.
SWEEPS1 = [(512, 512, [4, 5, 6, 7]), (0, 256, [0, 1]), (256, 256, [2, 3]), (1024, 256, [8, 9])]
# AllGather chunks, all 1MB: (m-tile base, n m-tiles).  The first sweep
# ships as TWO chunks so layer-2 pass 1 can start on an early small chunk
# instead of waiting out a 2MB mesh.
AG_CHUNKS = [(4, 2), (6, 2), (0, 2), (2, 2), (8, 2)]

_NTFF_HOOK_INSTALLED = False


def install_ntff_hook():
    """bass_utils' trace=True path wants antenv.axon_hooks; this container
    doesn't ship it, so provide the same ctypes hook trn_boot would."""
    global _NTFF_HOOK_INSTALLED
    if _NTFF_HOOK_INSTALLED:
        return
    _NTFF_HOOK_INSTALLED = True
    try:
        lib = ctypes.CDLL("/opt/axon/libaxon_pjrt.so")
        if not hasattr(lib, "axon_start_nrt_profile"):
            return
    except OSError:
        return
    lib.axon_start_nrt_profile.argtypes = [
        ctypes.POINTER(ctypes.c_int64),
        ctypes.c_size_t,
    ]
    lib.axon_start_nrt_profile.restype = ctypes.c_int64
    lib.axon_stop_nrt_profile.argtypes = [ctypes.c_char_p]
    lib.axon_stop_nrt_profile.restype = ctypes.c_int64

    @contextlib.contextmanager
    def _hook(output_dir, device_ids):
        import jax

        jax.devices()
        if device_ids:
            ids = (ctypes.c_int64 * len(device_ids))(*device_ids)
            rc = lib.axon_start_nrt_profile(ids, len(device_ids))
        else:
            rc = lib.axon_start_nrt_profile(None, 0)
        if rc != 0:
            raise RuntimeError(f"axon_start_nrt_profile rc={rc}")
        try:
            yield
        finally:
            n = lib.axon_stop_nrt_profile(str(output_dir).encode())
            print(f"ntff profile: {n} file(s) -> {output_dir}", file=sys.stderr)

    import antenv

    mod = types.ModuleType("antenv.axon_hooks")
    mod.get_axon_ntff_profile_hook = lambda: _hook
    mod.set_axon_ntff_profile_hook = lambda h: None
    sys.modules["antenv.axon_hooks"] = mod
    antenv.axon_hooks = mod


def split_drain_waits(nc):
    """This walrus build allows only ONE sync-wait per lowered instruction
    (CTRL and pseudo-DMA structs assert on more).  Tile's wait-assignment can
    attach several; keep the last wait on the instruction and move the rest
    onto preceding single-wait NoOps on the same engine stream (waits are
    monotonic >= conditions, so enforcing them earlier in program order on
    the same engine is equivalent)."""
    for f in nc.m.functions:
        for bb in f.blocks:
            insts = bb.instructions
            i = 0
            while i < len(insts):
                inst = insts[i]
                si = getattr(inst, "sync_info", None)
                if si is not None and si.on_wait and len(si.on_wait) > 1:
                    waits = list(si.on_wait)
                    si.on_wait = [waits[-1]]
                    for j, w in enumerate(waits[:-1]):
                        pre = mybir.InstNoOp(
                            name=f"{inst.name}-presync-{j}",
                            engine=inst.engine,
                            ins=[],
                            outs=[],
                            sync_info=mybir.SyncInfo(on_wait=[w], on_update=[]),
                        )
                        insts.insert(i + j, pre)
                        nc.register_instruction(pre, overwrite=True)
                    i += len(waits) - 1
                i += 1


def build_gcn(nc):
    """Emit the SPMD GCN program (identical on every core; per-core data)."""
    f32 = mybir.dt.float32
    bf16 = mybir.dt.bfloat16
    fp8 = mybir.dt.float8e4
    rg = [list(range(N_CORES))]

    # I/O (per-core shards; same names on every core).  All tile loads are
    # packed with >=1KB contiguous per partition to keep the DMA descriptor
    # count low on the chip-shared SDMA engines.
    # xpair[jp, p, q, c] = x[(2jp+q)*128+p, c]  (sweep lhsT pair tiles)
    xpair = nc.declare_dram_parameter("xpair", [NJP, P, 2, F_IN], fp8, isOutput=False)
    # ATr[g, p, u, q, c] = AhatT[(2g+u)*256+q*128+p, kR + 512 + c] (cols 512:1024)
    ATr = nc.declare_dram_parameter("ATr", [NJP // 2, P, 2, 2, 512], fp8, isOutput=False)
    # ATc1[jp, p, q, c] = AhatT[jp*256+q*128+p, kR + c]            (cols 0:512)
    # (used by layer-2 pass 1's SBUF refill; layer-1 streams the paired
    # halves ATc1a/ATc1b below instead)
    ATc1 = nc.declare_dram_parameter("ATc1", [NJP, P, 2, 512], fp8, isOutput=False)
    # ATc1a/b[g, p, u, q, c] = AhatT[(2g+u)*256+q*128+p, kR + off + c],
    # off = 0 / 256 (paired 2-jp blocks for 1KB DMA lines)
    ATc1a = nc.declare_dram_parameter("ATc1a", [NJP // 2, P, 2, 2, 256], fp8, isOutput=False)
    ATc1b = nc.declare_dram_parameter("ATc1b", [NJP // 2, P, 2, 2, 256], fp8, isOutput=False)
    # ATc2[g, p, u, q, c] = AhatT[(2g+u)*256+q*128+p, kR + 1024 + c] (cols 1024:1280)
    ATc2 = nc.declare_dram_parameter("ATc2", [NJP // 2, P, 2, 2, 256], fp8, isOutput=False)
    # W pair layouts: W*p8[t, p, q, n] = W[(2t+q)*128 + p, n]
    W1p = nc.declare_dram_parameter("W1p", [NFP, P, 2, F_HID], fp8, isOutput=False)
    W2p = nc.declare_dram_parameter("W2p", [NFP, P, 2, F_HID], fp8, isOutput=False)
    Woutp = nc.declare_dram_parameter("Woutp", [NFP, P, 2, N_CLASSES], fp8, isOutput=False)
    bcols = nc.declare_dram_parameter("bcols", [P, 2 * NF], f32, isOutput=False)
    bout = nc.declare_dram_parameter("bout", [1, N_CLASSES], bf16, isOutput=False)
    # out[p, m*16+c] = prob(node m*128+p, class c); host re-interleaves.
    out = nc.declare_dram_parameter("out", [P, NM * N_CLASSES], f32, isOutput=True)

    # layer-2 collective bounce buffers (internal DRAM), fp8
    ag_in = nc.dram_tensor("ag_in", [R, F_HID], fp8)
    ag_out = [
        nc.dram_tensor(
            f"ag_out{c}", [N_CORES * nt * P, F_HID], fp8, addr_space="Shared"
        )
        for c, (_b, nt) in enumerate(AG_CHUNKS)
    ]
    # warm-up collective buffers (uninitialized garbage).  ncfw pays a
    # ~20-70us wake-from-idle latency on any collective whose doorbell rings
    # while it is idle; queued ones start in ~2us.  Two small dummy
    # AllGathers ring at ~17us so the wake happens off the critical path.
    N_WARM = 2
    AGW = P
    agw_in = nc.dram_tensor("agw_in", [AGW, F_HID], fp8)
    agw_out = [
        nc.dram_tensor(f"agw_out{w}", [N_CORES * AGW, F_HID], fp8, addr_space="Shared")
        for w in range(N_WARM)
    ]

    with tile.TileContext(nc) as tc:
        with (
            tc.tile_pool(name="const", bufs=1) as cpool,
            tc.tile_pool(name="tfull", bufs=1) as tpool,
            tc.tile_pool(name="hT", bufs=1) as hpool,
            tc.tile_pool(name="atB", bufs=1) as bpool,
            tc.tile_pool(name="work", bufs=8) as wpool,
            tc.tile_pool(name="evac", bufs=4) as epool,
            tc.tile_pool(name="sm", bufs=4) as spool,
            tc.tile_pool(name="psum", bufs=1, space="PSUM") as ppool,
        ):
            # ---- collective warm-up (see agw_out comment) ----
            for w in range(N_WARM):
                nc.gpsimd.collective_compute(
                    "AllGather",
                    mybir.AluOpType.bypass,
                    replica_groups=rg,
                    ins=[agw_in[:, :].opt()],
                    outs=[agw_out[w][:, :].opt()],
                )

            # ---- layer 1 sweep input: x pair tiles straight from DRAM ----
            # (GCN is associative: h1 = (Ahat @ x) @ W1, so a replicated
            # x@W1 GEMM is replaced by small per-chunk W1 GEMMs on the
            # aggregated y = x^T A^T -- see w1_gemm below.)  Consts needed
            # early (W1/W2/bcols) are slotted into the stream so they land
            # before first use without delaying the sweep tail.
            t_pair = [
                [
                    tpool.tile([P, 2, F_HID], fp8, tag=f"tp{s}_{jp}", name=f"tp{s}_{jp}")
                    for jp in range(NJP)
                ]
                for s in range(2)
            ]
            # consts go AFTER all xpair tiles: the first sweep consumes
            # xpair at 0.855us/jp vs the queue's ~0.65us/jp dual-stream
            # delivery, and a mid-stream const burst showed up verbatim as
            # a tensor gap.  W1 (first use ~58us) still lands by ~42us.
            for jp in range(NJP):
                nc.sync.dma_start(out=t_pair[0][jp][:], in_=xpair[jp, :, :, :])
            W1_sb = [cpool.tile([P, 2, F_HID], fp8, tag=f"W1{t}", name=f"W1{t}") for t in range(NFP)]
            for t in range(NFP):
                nc.sync.dma_start(out=W1_sb[t][:], in_=W1p[t, :, :, :])
            W2_sb = [cpool.tile([P, 2, F_HID], fp8, tag=f"W2{t}", name=f"W2{t}") for t in range(NFP)]
            for t in range(NFP):
                nc.sync.dma_start(out=W2_sb[t][:], in_=W2p[t, :, :, :])
            bcols_sb = cpool.tile([P, 2 * NF], f32, tag="bcols", name="bcols")
            nc.sync.dma_start(out=bcols_sb[:], in_=bcols[:, :])
            Wout_sb = [cpool.tile([P, 2, N_CLASSES], fp8, tag=f"Wo{t}", name=f"Wo{t}") for t in range(NFP)]
            for t in range(NFP):
                nc.sync.dma_start(out=Wout_sb[t][:], in_=Woutp[t, :, :, :])
            bout_sb = cpool.tile([1, N_CLASSES], bf16, tag="bout", name="bout")
            nc.sync.dma_start(out=bout_sb[:], in_=bout[:, :])
            ones_sb = cpool.tile([1, P], bf16, tag="ones", name="ones")
            nc.vector.memset(ones_sb[:], 1.0)
            outsb = cpool.tile([P, NM * N_CLASSES], f32, tag="outsb", name="outsb")

            # persistent activation state
            # hT as fp8 feature-pair tiles: hp[layer][t][p, q, m],
            # f-tile index ft = 2t+q  (directly the next GEMM's lhsT pairs)
            hp = [
                [hpool.tile([P, 2, R], fp8, tag=f"h{la}p{t}", name=f"h{la}p{t}") for t in range(NFP)]
                for la in range(2)
            ]
            # y = x^T A^T pair tiles (pre-W1 aggregate, layer 1 only)
            yp = [hpool.tile([P, 2, R], fp8, tag=f"yp{t}", name=f"yp{t}") for t in range(NFP)]

            # resident adjacency (cols 512:1024): JIT-streamed by layer-1's
            # first sweep, kept in SBUF, reused by layer-2's R sweep.
            atr_sb = [
                bpool.tile([P, 2, 2, 512], fp8, tag=f"atr{g}", name=f"atr{g}")
                for g in range(NJP // 2)
            ]

            def get_sw(si):
                """Layer-1 sweep si: per-jp adjacency accessor (JIT stream)."""
                if si == 0:
                    def get(jp):
                        g, u = jp // 2, jp % 2
                        if u == 0:
                            # scalar queue; the sync queue carries xpair.
                            # (The gpsimd/SWDGE queue is NOT an option for
                            # this stream: +40us measured.)
                            nc.scalar.dma_start(out=atr_sb[g][:], in_=ATr[g, :, :, :, :])
                        return atr_sb[g][:, u, :, :]
                else:
                    src = [None, ATc1a, ATc1b, ATc2][si]

                    def get(jp, src=src, si=si):
                        g, u = jp // 2, jp % 2
                        if u == 0:
                            at = wpool.tile([P, 2, 2, 256], fp8, tag=f"atw{si % 2}", name=f"atw{si}_{g}")
                            nc.scalar.dma_start(out=at[:], in_=src[g, :, :, :, :])
                            get._cur = at
                        return get._cur[:, u, :, :]
                return get

            def get_r2(jp):
                return atr_sb[jp // 2][:, jp % 2, :, :]

            def w1_gemm(off, width, pstag):
                """hp[0][.., off:off+width] = relu(W1^T y + b1)."""
                for ft in range(NF):
                    ps = ppool.tile([P, width], f32, tag=f"sp{pstag[ft]}", name=f"w1g{off}_{ft}")
                    for t in range(NFP):
                        nc.tensor.matmul(
                            out=ps[:],
                            lhsT=W1_sb[t][:, :, ft * P:(ft + 1) * P],
                            rhs=yp[t][:, :, off:off + width],
                            start=(t == 0),
                            stop=(t == NFP - 1),
                            perf_mode=mybir.MatmulPerfMode.DoubleRow,
                        )
                    nc.vector.tensor_scalar(
                        out=hp[0][ft // 2][:, ft % 2, off:off + width],
                        in0=ps[:],
                        scalar1=bcols_sb[:, ft:ft + 1],
                        scalar2=0.0,
                        op0=mybir.AluOpType.add,
                        op1=mybir.AluOpType.max,
                    )

            def gemm2_tiles(ms, pstag):
                """t2_k rows for m-tiles `ms` staged into ag_in (as fp8)."""
                for i, m in enumerate(ms):
                    ps = ppool.tile([P, F_HID], f32, tag=f"sp{pstag[i % len(pstag)]}", name=f"g2ps{m}")
                    for t in range(NFP):
                        nc.tensor.matmul(
                            out=ps[:],
                            lhsT=hp[0][t][:, :, m * P:(m + 1) * P],
                            rhs=W2_sb[t][:, :, :],
                            start=(t == 0),
                            stop=(t == NFP - 1),
                            perf_mode=mybir.MatmulPerfMode.DoubleRow,
                        )
                    ev = epool.tile([P, F_HID], fp8, tag="g2ev", name="g2ev")
                    nc.vector.tensor_copy(out=ev[:], in_=ps[:])
                    # scalar queue: lands right behind the adjacency loads;
                    # the sync queue carries the AG-gated t2 reloads instead.
                    nc.scalar.dma_start(out=ag_in[m * P:(m + 1) * P, :], in_=ev[:])

            def ag_chunk(c):
                base, nt = AG_CHUNKS[c]
                nc.gpsimd.collective_compute(
                    "AllGather",
                    mybir.AluOpType.bypass,
                    replica_groups=rg,
                    ins=[ag_in[base * P:(base + nt) * P, :].opt()],
                    outs=[ag_out[c][:, :].opt()],
                )

            def load_t2_chunk(c):
                # ag_out[c] row (r*nt+i)*128 -> j = r*10 + base + i
                # (scalar queue: its adjacency stream is drained by the time
                # the first mesh completes, so these start the instant each
                # AllGather lands -- on the sync queue they'd sit behind the
                # 5.24MB ATc1 refill, delaying pass 1 by ~11us measured.
                # The per-chunk AG gates match arrival order, so they can't
                # head-of-line-block each other or the later atc2b stream.)
                base, nt = AG_CHUNKS[c]
                for r in range(N_CORES):
                    for i in range(nt):
                        j = r * NM + base + i
                        row = (r * nt + i) * P
                        nc.scalar.dma_start(
                            out=t_pair[1][j // 2][:, j % 2, :],
                            in_=ag_out[c][row:row + P, :],
                        )

            def head_tiles(ms):
                """logits + softmax for node tiles `ms` -> outsb columns."""
                for m in ms:
                    ps = ppool.tile([P, N_CLASSES], f32, tag=f"sp{4 + m % 4}", name=f"hps{m % 4}")
                    for t in range(NFP):
                        nc.tensor.matmul(
                            out=ps[:],
                            lhsT=hp[1][t][:, :, m * P:(m + 1) * P],
                            rhs=Wout_sb[t][:, :, :],
                            start=(t == 0),
                            stop=False,
                            perf_mode=mybir.MatmulPerfMode.DoubleRow,
                        )
                    nc.tensor.matmul(
                        out=ps[:],
                        lhsT=ones_sb[:, 0:P],
                        rhs=bout_sb[:],
                        start=False,
                        stop=True,
                    )
                    negmax = spool.tile([P, 1], f32, tag="negmax", name="negmax")
                    nc.vector.tensor_reduce(
                        out=negmax[:], in_=ps[:], axis=mybir.AxisListType.X,
                        op=mybir.AluOpType.max, negate=True,
                    )
                    ex = spool.tile([P, N_CLASSES], f32, tag="ex", name="ex")
                    nc.scalar.activation(
                        out=ex[:], in_=ps[:],
                        func=mybir.ActivationFunctionType.Exp,
                        bias=negmax[:, 0:1],
                    )
                    ssum = spool.tile([P, 1], f32, tag="ssum", name="ssum")
                    nc.vector.tensor_reduce(
                        out=ssum[:], in_=ex[:], axis=mybir.AxisListType.X,
                        op=mybir.AluOpType.add,
                    )
                    rinv = spool.tile([P, 1], f32, tag="rinv", name="rinv")
                    nc.vector.reciprocal(out=rinv[:], in_=ssum[:])
                    nc.vector.tensor_scalar_mul(
                        outsb[:, m * N_CLASSES:(m + 1) * N_CLASSES], ex[:], rinv[:, 0:1]
                    )

            # ---- layer 1: three sweeps, each shipping an AG chunk.
            # Sweeps alternate psum tag sets A=(0..3)/B=(4..7); each sweep's
            # trailing gemms use its own (freed) set and are emitted after
            # the NEXT sweep's first j-pairs so evacuation never bubbles.
            TAGSETS = [[0, 1, 2, 3], [4, 5, 6, 7]]

            pending = None  # (sweep index, tagset) awaiting gemms + AG
            for si, (off, width, mts) in enumerate(SWEEPS1):
                get = get_sw(si)
                ts = TAGSETS[si % 2]
                ps = [
                    ppool.tile([P, width], f32, tag=f"sp{ts[f]}", name=f"s1_{si}_{f}")
                    for f in range(NF)
                ]
                for jp in range(NJP):
                    at = get(jp)
                    for f in range(NF):
                        nc.tensor.matmul(
                            out=ps[f][:],
                            lhsT=t_pair[0][jp][:, :, f * P:(f + 1) * P],
                            rhs=at,
                            start=(jp == 0),
                            stop=(jp == NJP - 1),
                            perf_mode=mybir.MatmulPerfMode.DoubleRow,
                        )
                    if jp == 3 and pending is not None:
                        po, pts = pending
                        poff, pw, pmts = SWEEPS1[po]
                        w1_gemm(poff, pw, pts)
                        for c, (cb, cn) in enumerate(AG_CHUNKS):
                            if cb in pmts:
                                gemm2_tiles(list(range(cb, cb + cn)), pts)
                                ag_chunk(c)
                        pending = None
                for f in range(NF):
                    nc.vector.tensor_copy(
                        out=yp[f // 2][:, f % 2, off:off + width],
                        in_=ps[f][:],
                    )
                pending = (si, ts)
            # last sweep's gemms + chunk (nothing left to hide them under)
            po, pts = pending
            poff, pw, pmts = SWEEPS1[po]
            w1_gemm(poff, pw, pts)
            for c, (cb, cn) in enumerate(AG_CHUNKS):
                if cb in pmts:
                    gemm2_tiles(list(range(cb, cb + cn)), pts)
                    ag_chunk(c)
            # x tiles are dead after the last sweep; refill them with ATc1
            # (same [P, 2, 512] shape) on the idle sync queue so layer-2's
            # pass 1 reads BOTH adjacency halves from SBUF -- streaming it
            # JIT under the resident atr reads cost ~24us of SBUF write/read
            # port contention (measured pass-1 at 92us vs 68us model).
            # Each refill DMA waits only on that tile's last sweep read, so
            # the stream rides right behind the final sweep's progress and
            # finishes during the pre-pass-1 AllGather wait; the chunk-gated
            # t2 reloads are emitted behind it and start at ~the same time.
            for jp in range(NJP):
                nc.sync.dma_start(out=t_pair[0][jp][:], in_=ATc1[jp, :, :, :])
            for c in range(len(AG_CHUNKS)):
                load_t2_chunk(c)

            # ---- layer 2: consume j-pairs in AG-chunk arrival order ----
            order2 = [
                5 * r + base // 2 + ii
                for (base, nt) in AG_CHUNKS
                for r in range(N_CORES)
                for ii in range(nt // 2)
            ]
            assert sorted(order2) == list(range(NJP))

            # Pass 1: cols 0:1024 -- streamed ATc1 (re-read) into banks 0-3
            # plus resident atr into banks 4-7; all 8 PSUM banks live.  This
            # puts 2/3 of layer-2 in the first pass so only the short 256-col
            # pass 2 (plus the softmax heads) remains after the last
            # AllGather chunk's j-pairs -- the post-wire tail drops from
            # ~55us to ~38us.
            psa = [ppool.tile([P, 512], f32, tag=f"sp{f}", name=f"s2a_{f}") for f in range(NF)]
            psb = [ppool.tile([P, 512], f32, tag=f"sp{4 + f}", name=f"s2b_{f}") for f in range(NF)]
            for idx, jp in enumerate(order2):
                at = t_pair[0][jp]
                ar = get_r2(jp)
                for f in range(NF):
                    nc.tensor.matmul(
                        out=psa[f][:],
                        lhsT=t_pair[1][jp][:, :, f * P:(f + 1) * P],
                        rhs=at[:, :, :],
                        start=(idx == 0),
                        stop=(idx == NJP - 1),
                        perf_mode=mybir.MatmulPerfMode.DoubleRow,
                    )
                    nc.tensor.matmul(
                        out=psb[f][:],
                        lhsT=t_pair[1][jp][:, :, f * P:(f + 1) * P],
                        rhs=ar[:, :, :],
                        start=(idx == 0),
                        stop=(idx == NJP - 1),
                        perf_mode=mybir.MatmulPerfMode.DoubleRow,
                    )
            for f in range(NF):
                nc.vector.tensor_scalar(
                    out=hp[1][f // 2][:, f % 2, 0:512],
                    in0=psa[f][:],
                    scalar1=bcols_sb[:, NF + f:NF + f + 1],
                    scalar2=0.0,
                    op0=mybir.AluOpType.add,
                    op1=mybir.AluOpType.max,
                )
                nc.vector.tensor_scalar(
                    out=hp[1][f // 2][:, f % 2, 512:1024],
                    in0=psb[f][:],
                    scalar1=bcols_sb[:, NF + f:NF + f + 1],
                    scalar2=0.0,
                    op0=mybir.AluOpType.add,
                    op1=mybir.AluOpType.max,
                )

            # Pass 2: cols 1024:1280, two column-sliced [P, 512] psum tiles
            # (banks 0-1); heads for m0..7 overlap it on banks 4-7.  Pass 2
            # runs entirely after pass 1 (whose tail needed the last AG
            # chunk), so every j-pair is present -- natural order is fine
            # and keeps the packed paired ATc2 loads.
            ps2 = [ppool.tile([P, 512], f32, tag=f"sp{t}", name=f"s2c_{t}") for t in range(NFP)]
            for jp in range(NJP):
                g, u = jp // 2, jp % 2
                if u == 0:
                    at2 = wpool.tile([P, 2, 2, 256], fp8, tag="atc2b", name=f"atc2b_{g}")
                    nc.scalar.dma_start(out=at2[:], in_=ATc2[g, :, :, :, :])
                    cur2 = at2
                for f in range(NF):
                    nc.tensor.matmul(
                        out=ps2[f // 2][:, (f % 2) * 256:(f % 2) * 256 + 256],
                        lhsT=t_pair[1][jp][:, :, f * P:(f + 1) * P],
                        rhs=cur2[:, u, :, :],
                        start=(jp == 0),
                        stop=(jp == NJP - 1),
                        perf_mode=mybir.MatmulPerfMode.DoubleRow,
                    )
                if jp == 3:
                    head_tiles([0, 1, 2, 3, 4, 5, 6, 7])
                if jp == 20:
                    # ship the finished heads' output columns during pass 2
                    # so only m8/m9's 32KB rides the final DMA
                    nc.sync.dma_start(
                        out=out[:, 0:8 * N_CLASSES], in_=outsb[:, 0:8 * N_CLASSES]
                    )
            for f in range(NF):
                nc.vector.tensor_scalar(
                    out=hp[1][f // 2][:, f % 2, 1024:1280],
                    in0=ps2[f // 2][:, (f % 2) * 256:(f % 2) * 256 + 256],
                    scalar1=bcols_sb[:, NF + f:NF + f + 1],
                    scalar2=0.0,
                    op0=mybir.AluOpType.add,
                    op1=mybir.AluOpType.max,
                )
            head_tiles([8, 9])

            nc.sync.dma_start(
                out=out[:, 8 * N_CLASSES:], in_=outsb[:, 8 * N_CLASSES:]
            )

    return nc


def build_inputs(x, edge_index, W1, b1, W2, b2, Wout, bout):
    """Host-side graph preprocessing + per-core shard construction."""
    x = np.asarray(x)
    ei = np.asarray(edge_index)
    n = N_NODES
    src = np.concatenate([ei[0], np.arange(n, dtype=np.int64)])
    dst = np.concatenate([ei[1], np.arange(n, dtype=np.int64)])
    deg = np.bincount(dst, minlength=n).astype(np.float32)
    dinv = 1.0 / np.sqrt(deg)
    normv = (dinv[src] * dinv[dst]).astype(np.float32)

    # dense Ahat^T, padded:  AhatT[src, dst] = norm  (duplicate edges sum)
    AhatT = np.zeros((NP, NP), dtype=np.float32)
    np.add.at(AhatT, (src, dst), normv)
    # DoubleRow pair-interleave: blocks[jp, p, q, :] = AhatT[jp*256+q*128+p, :]
    blocks = AhatT.astype(FP8).reshape(NJP, 2, P, NP).transpose(0, 2, 1, 3)

    xp = np.zeros((NP, F_IN), dtype=np.float32)
    xp[:n] = x
    # xpair[jp, p, q, c] = x[(2jp+q)*128+p, c]
    xpair = np.ascontiguousarray(
        xp.reshape(NJP, 2, P, F_IN).transpose(0, 2, 1, 3)
    ).astype(FP8)

    def wpairs(W):
        W = np.asarray(W, np.float32)
        # [t, p, q, n] = W[(2t+q)*128+p, n]
        return np.ascontiguousarray(
            W.reshape(NFP, 2, P, W.shape[1]).transpose(0, 2, 1, 3)
        ).astype(FP8)

    W1b = wpairs(W1)
    W2b = wpairs(W2)
    Woutb = wpairs(Wout)
    boutb = np.asarray(bout).reshape(1, N_CLASSES).astype(BF16)
    # biases as per-partition columns: bcols[:, l*NF + f] = b_l[f*128:(f+1)*128]
    bcols = np.stack(
        [np.asarray(b1).reshape(NF, P), np.asarray(b2).reshape(NF, P)], 0
    ).reshape(2 * NF, P).T.astype(np.float32)
    bcols = np.ascontiguousarray(bcols)

    in_maps = []
    for k in range(N_CORES):
        # blkp[g, p, u, q, :] = AhatT row (2g+u)*256+q*128+p, this core's cols
        blkp = blocks[:, :, :, k * R:(k + 1) * R].reshape(
            NJP // 2, 2, P, 2, R
        ).transpose(0, 2, 1, 3, 4)
        ATr_ = np.ascontiguousarray(blkp[..., 512:1024])
        ATc2_ = np.ascontiguousarray(blkp[..., 1024:1280])
        ATc1a_ = np.ascontiguousarray(blkp[..., 0:256])
        ATc1b_ = np.ascontiguousarray(blkp[..., 256:512])
        blk = blocks[:, :, :, k * R:(k + 1) * R]
        ATc1_ = np.ascontiguousarray(blk[..., 0:512])
        in_maps.append({
            "xpair": xpair,
            "ATr": ATr_,
            "ATc1": ATc1_,
            "ATc1a": ATc1a_,
            "ATc1b": ATc1b_,
            "ATc2": ATc2_,
            "W1p": W1b,
            "W2p": W2b,
            "Woutp": Woutb,
            "bcols": bcols,
            "bout": boutb,
        })
    return in_maps


_CACHED = {}


def _get_program():
    if "nc" not in _CACHED:
        nc = bass.Bass(num_devices=N_CORES)
        build_gcn(nc)
        split_drain_waits(nc)
        _CACHED["nc"] = nc
    return _CACHED["nc"]


def kernel(x, edge_index, W1, b1, W2, b2, Wout, bout, trace=False):
    install_ntff_hook()
    nc = _get_program()
    in_maps = build_inputs(x, edge_index, W1, b1, W2, b2, Wout, bout)
    res = run_bass_kernel_spmd(
        nc, in_maps, core_ids=list(range(N_CORES)), trace=trace
    )
    # out[p, m*16+c] -> rows m*128+p
    outs = []
    for k in range(N_CORES):
        o = res.results[k]["out"]
        outs.append(o.reshape(P, NM, N_CLASSES).transpose(1, 0, 2).reshape(R, N_CLASSES))
    out = np.concatenate(outs, 0)
    kernel.last_exec_time_ns = res.exec_time_ns
    kernel.last_results = res
    return out[:N_NODES].astype(np.float32)


kernel.last_exec_time_ns = None
kernel.last_results = None


# revision 31
# speedup vs baseline: 1.0421x; 1.0421x over previous
"""Trainium2 8-core GCN kernel (2-layer GCNConv + linear head + softmax).

Strategy (node/row partitioning, dense normalized adjacency):
  - Host: build Ahat = D^-1/2 (A+I) D^-1/2 as a dense fp8-e4m3 matrix, padded
    from 10000 to 10240 nodes; core k owns node rows [k*1280, (k+1)*1280).
  - Device, per core k (all matmuls fp8-e4m3 DoubleRow, fp32 accumulate):
      y_k    = x^T Ahat^T[:,k]  (transposed SpMM, swept in column chunks)
      h1_k   = relu(W1^T y_k + b1)          (associativity: (A x) W1)
      t2_k   = (h1_k)^T @ W2                (h1T is directly the lhsT)
      t2     = AllGather(t2_k), one chunk per layer-1 sweep (2MB+2MB+1MB)
      h2T_k  = relu(t2^T Ahat^T[:,k] + b2)
      out_k  = softmax(h2T_k^T @ Wout + bout) ([1280, 16] f32)
  - Host: concatenate core outputs, trim padding to [10000, 16].

Schedule (what makes this fast; measured on HW):
  - The cores launch with a consistent ~40-55us stagger (constant per-chunk
    doorbell lag across the run -- an offset, not a clock-rate skew), and
    ncfw (the collective firmware, a SERIAL engine) adds a ~60us
    wake-from-idle before its first mesh; queued doorbells start in ~2us.
    Two small dummy AllGathers ring at ~15us so the wake burns off the
    critical path; each real 1MB chunk costs ~13-21us of mesh service
    (mostly slowest-peer waits inside).
  - Layer 1 runs four column sweeps -- 512:1024 (512-wide), 0:256,
    256:512, 1024:1280 (256-wide) -- each followed by its W1 GEMM,
    t2 GEMM, and 1MB AllGather chunk(s) (the first sweep ships two), so
    doorbells ring every ~20us from ~60us on and the 5.24MB wire runs as
    a continuously-fed conveyor, finishing ~188us.  The FIRST sweep must
    be 512-wide: its xpair+ATr co-load paces at the shared-SDMA ~220GB/s
    ceiling, and a narrow first sweep stretches 2x.  (Going narrower
    everywhere is strictly worse: ~46ns fixed cost per matmul, measured
    +36us tensor-busy at width-256 everywhere.)
  - Const loads go AFTER the whole xpair stream (a mid-stream burst shows
    up 1:1 as a tensor gap); each sweep's trailing GEMMs are emitted after
    the NEXT sweep's first j-pairs (alternating PSUM tag sets) so psum
    evacuation never bubbles the tensor queue.
  - Layer 2 runs as pass 1 = cols 0:1024 (all 8 PSUM banks: streamed
    0:512 + resident 512:1024) consuming j-pairs in chunk-arrival order,
    then a short pass 2 = cols 1024:1280 -- this puts only ~14us of
    pass-1 tail plus pass 2 (~21us) after the last chunk's data, vs ~50us
    for an even S/R split.  Heads for m0..7 overlap pass 2.
  - Layer-2 pass 1 reads BOTH adjacency halves from SBUF: cols 512:1024
    are resident from layer-1's first sweep, and cols 0:512 are refilled
    into the dead x tiles (same shape) on the idle sync queue -- streaming
    them JIT under the resident reads cost ~20us of SBUF port contention.
  - t2 reloads ride the scalar queue (drained by the time each mesh
    lands); on the sync queue they'd sit behind the 5.24MB refill and
    delay pass 1 by ~11us.  Their per-chunk AllGather gates match arrival
    order, so they cannot head-of-line-block each other.

The transposed SpMM (z^T = t^T A^T instead of z = A t) makes each layer's
activation land in [feature, node] layout, which is exactly the lhsT the
following GEMM needs -- no on-device transposes anywhere.  All matmuls use
perf_mode=DoubleRow (256 contraction rows per matmul): lhsT/rhs are
[128, 2, free] pair tiles, element [p, q] = contraction row q*128+p.
"""

import contextlib
import ctypes
import sys
import types

import ml_dtypes
import numpy as np

import concourse.bass as bass
import concourse.mybir as mybir
import concourse.tile as tile
from concourse.bass_utils import run_bass_kernel_spmd

BF16 = ml_dtypes.bfloat16
FP8 = ml_dtypes.float8_e4m3

N_CORES = 8
N_NODES = 10000
F_IN = 512
F_HID = 512
N_CLASSES = 16
NP = 10240            # padded node count (80 * 128)
R = NP // N_CORES     # 1280 rows per core
P = 128
NJ = NP // P          # 80 contraction chunks
NJP = NJ // 2         # 40 DoubleRow contraction pairs
NM = R // P           # 10 row tiles per core
NF = F_HID // P       # 4 feature tiles
NFP = NF // 2         # 2 feature pairs

# Layer-1 sweeps in execution order: (col offset, width, m-tiles).  The
# first sweep is 512-wide (its xpair+ATr co-load paces at the shared-SDMA
# ~220GB/s ceiling -- a narrow first sweep would stretch 2x); the rest are
# 256-wide so the AllGather conveyor gets doorbells every ~20us and each
# chunk's (arrival + remaining-pass-1-work) is flat across chunks.
SWEEPS1 = [(512, 512, [4, 5, 6, 7]), (0, 256, [0, 1]), (256, 256, [2, 3]), (1024, 256, [8, 9])]
# AllGather chunks: (m-tile base, n m-tiles).  Front-loaded: the first
# chunk is 2MB (grants 16 j-pairs of pass-1 work in one mesh) and the rest
# are 1MB, so chunk c's (arrival + remaining-pass-1-work) is flat across
# chunks.  Chunks must cover whole j-PAIRS (2 m-tiles): a half-pair chunk
# is useless, the DoubleRow matmul needs both halves.
AG_CHUNKS = [(4, 4), (0, 2), (2, 2), (8, 2)]

_NTFF_HOOK_INSTALLED = False


def install_ntff_hook():
    """bass_utils' trace=True path wants antenv.axon_hooks; this container
    doesn't ship it, so provide the same ctypes hook trn_boot would."""
    global _NTFF_HOOK_INSTALLED
    if _NTFF_HOOK_INSTALLED:
        return
    _NTFF_HOOK_INSTALLED = True
    try:
        lib = ctypes.CDLL("/opt/axon/libaxon_pjrt.so")
        if not hasattr(lib, "axon_start_nrt_profile"):
            return
    except OSError:
        return
    lib.axon_start_nrt_profile.argtypes = [
        ctypes.POINTER(ctypes.c_int64),
        ctypes.c_size_t,
    ]
    lib.axon_start_nrt_profile.restype = ctypes.c_int64
    lib.axon_stop_nrt_profile.argtypes = [ctypes.c_char_p]
    lib.axon_stop_nrt_profile.restype = ctypes.c_int64

    @contextlib.contextmanager
    def _hook(output_dir, device_ids):
        import jax

        jax.devices()
        if device_ids:
            ids = (ctypes.c_int64 * len(device_ids))(*device_ids)
            rc = lib.axon_start_nrt_profile(ids, len(device_ids))
        else:
            rc = lib.axon_start_nrt_profile(None, 0)
        if rc != 0:
            raise RuntimeError(f"axon_start_nrt_profile rc={rc}")
        try:
            yield
        finally:
            n = lib.axon_stop_nrt_profile(str(output_dir).encode())
            print(f"ntff profile: {n} file(s) -> {output_dir}", file=sys.stderr)

    import antenv

    mod = types.ModuleType("antenv.axon_hooks")
    mod.get_axon_ntff_profile_hook = lambda: _hook
    mod.set_axon_ntff_profile_hook = lambda h: None
    sys.modules["antenv.axon_hooks"] = mod
    antenv.axon_hooks = mod


def split_drain_waits(nc):
    """This walrus build allows only ONE sync-wait per lowered instruction
    (CTRL and pseudo-DMA structs assert on more).  Tile's wait-assignment can
    attach several; keep the last wait on the instruction and move the rest
    onto preceding single-wait NoOps on the same engine stream (waits are
    monotonic >= conditions, so enforcing them earlier in program order on
    the same engine is equivalent)."""
    for f in nc.m.functions:
        for bb in f.blocks:
            insts = bb.instructions
            i = 0
            while i < len(insts):
                inst = insts[i]
                si = getattr(inst, "sync_info", None)
                if si is not None and si.on_wait and len(si.on_wait) > 1:
                    waits = list(si.on_wait)
                    si.on_wait = [waits[-1]]
                    for j, w in enumerate(waits[:-1]):
                        pre = mybir.InstNoOp(
                            name=f"{inst.name}-presync-{j}",
                            engine=inst.engine,
                            ins=[],
                            outs=[],
                            sync_info=mybir.SyncInfo(on_wait=[w], on_update=[]),
                        )
                        insts.insert(i + j, pre)
                        nc.register_instruction(pre, overwrite=True)
                    i += len(waits) - 1
                i += 1


def build_gcn(nc):
    """Emit the SPMD GCN program (identical on every core; per-core data)."""
    f32 = mybir.dt.float32
    bf16 = mybir.dt.bfloat16
    fp8 = mybir.dt.float8e4
    rg = [list(range(N_CORES))]

    # I/O (per-core shards; same names on every core).  All tile loads are
    # packed with >=1KB contiguous per partition to keep the DMA descriptor
    # count low on the chip-shared SDMA engines.
    # xpair[jp, p, q, c] = x[(2jp+q)*128+p, c]  (sweep lhsT pair tiles)
    xpair = nc.declare_dram_parameter("xpair", [NJP, P, 2, F_IN], fp8, isOutput=False)
    # ATr[g, p, u, q, c] = AhatT[(2g+u)*256+q*128+p, kR + 512 + c] (cols 512:1024)
    ATr = nc.declare_dram_parameter("ATr", [NJP // 2, P, 2, 2, 512], fp8, isOutput=False)
    # ATc1[jp, p, q, c] = AhatT[jp*256+q*128+p, kR + c]            (cols 0:512)
    # (used by layer-2 pass 1's SBUF refill; layer-1 streams the paired
    # halves ATc1a/ATc1b below instead)
    ATc1 = nc.declare_dram_parameter("ATc1", [NJP, P, 2, 512], fp8, isOutput=False)
    # ATc1a/b[g, p, u, q, c] = AhatT[(2g+u)*256+q*128+p, kR + off + c],
    # off = 0 / 256 (paired 2-jp blocks for 1KB DMA lines)
    ATc1a = nc.declare_dram_parameter("ATc1a", [NJP // 2, P, 2, 2, 256], fp8, isOutput=False)
    ATc1b = nc.declare_dram_parameter("ATc1b", [NJP // 2, P, 2, 2, 256], fp8, isOutput=False)
    # ATc2[g, p, u, q, c] = AhatT[(2g+u)*256+q*128+p, kR + 1024 + c] (cols 1024:1280)
    ATc2 = nc.declare_dram_parameter("ATc2", [NJP // 2, P, 2, 2, 256], fp8, isOutput=False)
    # W pair layouts: W*p8[t, p, q, n] = W[(2t+q)*128 + p, n]
    W1p = nc.declare_dram_parameter("W1p", [NFP, P, 2, F_HID], fp8, isOutput=False)
    W2p = nc.declare_dram_parameter("W2p", [NFP, P, 2, F_HID], fp8, isOutput=False)
    Woutp = nc.declare_dram_parameter("Woutp", [NFP, P, 2, N_CLASSES], fp8, isOutput=False)
    bcols = nc.declare_dram_parameter("bcols", [P, 2 * NF], f32, isOutput=False)
    bout = nc.declare_dram_parameter("bout", [1, N_CLASSES], bf16, isOutput=False)
    # out[p, m*16+c] = prob(node m*128+p, class c); host re-interleaves.
    out = nc.declare_dram_parameter("out", [P, NM * N_CLASSES], f32, isOutput=True)

    # layer-2 collective bounce buffers (internal DRAM), fp8
    ag_in = nc.dram_tensor("ag_in", [R, F_HID], fp8)
    ag_out = [
        nc.dram_tensor(
            f"ag_out{c}", [N_CORES * nt * P, F_HID], fp8, addr_space="Shared"
        )
        for c, (_b, nt) in enumerate(AG_CHUNKS)
    ]
    # warm-up collective buffers (uninitialized garbage).  ncfw pays a
    # ~20-70us wake-from-idle latency on any collective whose doorbell rings
    # while it is idle; queued ones start in ~2us.  Two small dummy
    # AllGathers ring at ~17us so the wake happens off the critical path.
    N_WARM = 2
    AGW = P
    agw_in = nc.dram_tensor("agw_in", [AGW, F_HID], fp8)
    agw_out = [
        nc.dram_tensor(f"agw_out{w}", [N_CORES * AGW, F_HID], fp8, addr_space="Shared")
        for w in range(N_WARM)
    ]

    with tile.TileContext(nc) as tc:
        with (
            tc.tile_pool(name="const", bufs=1) as cpool,
            tc.tile_pool(name="tfull", bufs=1) as tpool,
            tc.tile_pool(name="hT", bufs=1) as hpool,
            tc.tile_pool(name="atB", bufs=1) as bpool,
            tc.tile_pool(name="work", bufs=8) as wpool,
            tc.tile_pool(name="evac", bufs=4) as epool,
            tc.tile_pool(name="sm", bufs=4) as spool,
            tc.tile_pool(name="psum", bufs=1, space="PSUM") as ppool,
        ):
            # ---- collective warm-up (see agw_out comment) ----
            for w in range(N_WARM):
                nc.gpsimd.collective_compute(
                    "AllGather",
                    mybir.AluOpType.bypass,
                    replica_groups=rg,
                    ins=[agw_in[:, :].opt()],
                    outs=[agw_out[w][:, :].opt()],
                )

            # ---- layer 1 sweep input: x pair tiles straight from DRAM ----
            # (GCN is associative: h1 = (Ahat @ x) @ W1, so a replicated
            # x@W1 GEMM is replaced by small per-chunk W1 GEMMs on the
            # aggregated y = x^T A^T -- see w1_gemm below.)  Consts needed
            # early (W1/W2/bcols) are slotted into the stream so they land
            # before first use without delaying the sweep tail.
            t_pair = [
                [
                    tpool.tile([P, 2, F_HID], fp8, tag=f"tp{s}_{jp}", name=f"tp{s}_{jp}")
                    for jp in range(NJP)
                ]
                for s in range(2)
            ]
            # consts go AFTER all xpair tiles: the first sweep consumes
            # xpair at 0.855us/jp vs the queue's ~0.65us/jp dual-stream
            # delivery, and a mid-stream const burst showed up verbatim as
            # a tensor gap.  W1 (first use ~58us) still lands by ~42us.
            for jp in range(NJP):
                nc.sync.dma_start(out=t_pair[0][jp][:], in_=xpair[jp, :, :, :])
            W1_sb = [cpool.tile([P, 2, F_HID], fp8, tag=f"W1{t}", name=f"W1{t}") for t in range(NFP)]
            for t in range(NFP):
                nc.sync.dma_start(out=W1_sb[t][:], in_=W1p[t, :, :, :])
            W2_sb = [cpool.tile([P, 2, F_HID], fp8, tag=f"W2{t}", name=f"W2{t}") for t in range(NFP)]
            for t in range(NFP):
                nc.sync.dma_start(out=W2_sb[t][:], in_=W2p[t, :, :, :])
            bcols_sb = cpool.tile([P, 2 * NF], f32, tag="bcols", name="bcols")
            nc.sync.dma_start(out=bcols_sb[:], in_=bcols[:, :])
            Wout_sb = [cpool.tile([P, 2, N_CLASSES], fp8, tag=f"Wo{t}", name=f"Wo{t}") for t in range(NFP)]
            for t in range(NFP):
                nc.sync.dma_start(out=Wout_sb[t][:], in_=Woutp[t, :, :, :])
            bout_sb = cpool.tile([1, N_CLASSES], bf16, tag="bout", name="bout")
            nc.sync.dma_start(out=bout_sb[:], in_=bout[:, :])
            ones_sb = cpool.tile([1, P], bf16, tag="ones", name="ones")
            nc.vector.memset(ones_sb[:], 1.0)
            outsb = cpool.tile([P, NM * N_CLASSES], f32, tag="outsb", name="outsb")

            # persistent activation state
            # hT as fp8 feature-pair tiles: hp[layer][t][p, q, m],
            # f-tile index ft = 2t+q  (directly the next GEMM's lhsT pairs)
            hp = [
                [hpool.tile([P, 2, R], fp8, tag=f"h{la}p{t}", name=f"h{la}p{t}") for t in range(NFP)]
                for la in range(2)
            ]
            # y = x^T A^T pair tiles (pre-W1 aggregate, layer 1 only)
            yp = [hpool.tile([P, 2, R], fp8, tag=f"yp{t}", name=f"yp{t}") for t in range(NFP)]

            # resident adjacency (cols 512:1024): JIT-streamed by layer-1's
            # first sweep, kept in SBUF, reused by layer-2's R sweep.
            atr_sb = [
                bpool.tile([P, 2, 2, 512], fp8, tag=f"atr{g}", name=f"atr{g}")
                for g in range(NJP // 2)
            ]

            def get_sw(si):
                """Layer-1 sweep si: per-jp adjacency accessor (JIT stream)."""
                if si == 0:
                    def get(jp):
                        g, u = jp // 2, jp % 2
                        if u == 0:
                            # scalar queue; the sync queue carries xpair.
                            # (The gpsimd/SWDGE queue is NOT an option for
                            # this stream: +40us measured.)
                            nc.scalar.dma_start(out=atr_sb[g][:], in_=ATr[g, :, :, :, :])
                        return atr_sb[g][:, u, :, :]
                else:
                    src = [None, ATc1a, ATc1b, ATc2][si]

                    def get(jp, src=src, si=si):
                        g, u = jp // 2, jp % 2
                        if u == 0:
                            at = wpool.tile([P, 2, 2, 256], fp8, tag=f"atw{si % 2}", name=f"atw{si}_{g}")
                            nc.scalar.dma_start(out=at[:], in_=src[g, :, :, :, :])
                            get._cur = at
                        return get._cur[:, u, :, :]
                return get

            def get_r2(jp):
                return atr_sb[jp // 2][:, jp % 2, :, :]

            def w1_gemm(off, width, pstag):
                """hp[0][.., off:off+width] = relu(W1^T y + b1)."""
                for ft in range(NF):
                    ps = ppool.tile([P, width], f32, tag=f"sp{pstag[ft]}", name=f"w1g{off}_{ft}")
                    for t in range(NFP):
                        nc.tensor.matmul(
                            out=ps[:],
                            lhsT=W1_sb[t][:, :, ft * P:(ft + 1) * P],
                            rhs=yp[t][:, :, off:off + width],
                            start=(t == 0),
                            stop=(t == NFP - 1),
                            perf_mode=mybir.MatmulPerfMode.DoubleRow,
                        )
                    nc.vector.tensor_scalar(
                        out=hp[0][ft // 2][:, ft % 2, off:off + width],
                        in0=ps[:],
                        scalar1=bcols_sb[:, ft:ft + 1],
                        scalar2=0.0,
                        op0=mybir.AluOpType.add,
                        op1=mybir.AluOpType.max,
                    )

            def gemm2_tiles(ms, pstag):
                """t2_k rows for m-tiles `ms` staged into ag_in (as fp8)."""
                for i, m in enumerate(ms):
                    ps = ppool.tile([P, F_HID], f32, tag=f"sp{pstag[i % len(pstag)]}", name=f"g2ps{m}")
                    for t in range(NFP):
                        nc.tensor.matmul(
                            out=ps[:],
                            lhsT=hp[0][t][:, :, m * P:(m + 1) * P],
                            rhs=W2_sb[t][:, :, :],
                            start=(t == 0),
                            stop=(t == NFP - 1),
                            perf_mode=mybir.MatmulPerfMode.DoubleRow,
                        )
                    ev = epool.tile([P, F_HID], fp8, tag="g2ev", name="g2ev")
                    nc.vector.tensor_copy(out=ev[:], in_=ps[:])
                    # scalar queue: lands right behind the adjacency loads;
                    # the sync queue carries the AG-gated t2 reloads instead.
                    nc.scalar.dma_start(out=ag_in[m * P:(m + 1) * P, :], in_=ev[:])

            def ag_chunk(c):
                base, nt = AG_CHUNKS[c]
                nc.gpsimd.collective_compute(
                    "AllGather",
                    mybir.AluOpType.bypass,
                    replica_groups=rg,
                    ins=[ag_in[base * P:(base + nt) * P, :].opt()],
                    outs=[ag_out[c][:, :].opt()],
                )

            def load_t2_chunk(c):
                # ag_out[c] row (r*nt+i)*128 -> j = r*10 + base + i
                # (scalar queue: its adjacency stream is drained by the time
                # the first mesh completes, so these start the instant each
                # AllGather lands -- on the sync queue they'd sit behind the
                # 5.24MB ATc1 refill, delaying pass 1 by ~11us measured.
                # The per-chunk AG gates match arrival order, so they can't
                # head-of-line-block each other or the later atc2b stream.)
                base, nt = AG_CHUNKS[c]
                for r in range(N_CORES):
                    for i in range(nt):
                        j = r * NM + base + i
                        row = (r * nt + i) * P
                        nc.scalar.dma_start(
                            out=t_pair[1][j // 2][:, j % 2, :],
                            in_=ag_out[c][row:row + P, :],
                        )

            def head_tiles(ms):
                """logits + softmax for node tiles `ms` -> outsb columns."""
                for m in ms:
                    ps = ppool.tile([P, N_CLASSES], f32, tag=f"sp{4 + m % 4}", name=f"hps{m % 4}")
                    for t in range(NFP):
                        nc.tensor.matmul(
                            out=ps[:],
                            lhsT=hp[1][t][:, :, m * P:(m + 1) * P],
                            rhs=Wout_sb[t][:, :, :],
                            start=(t == 0),
                            stop=False,
                            perf_mode=mybir.MatmulPerfMode.DoubleRow,
                        )
                    nc.tensor.matmul(
                        out=ps[:],
                        lhsT=ones_sb[:, 0:P],
                        rhs=bout_sb[:],
                        start=False,
                        stop=True,
                    )
                    negmax = spool.tile([P, 1], f32, tag="negmax", name="negmax")
                    nc.vector.tensor_reduce(
                        out=negmax[:], in_=ps[:], axis=mybir.AxisListType.X,
                        op=mybir.AluOpType.max, negate=True,
                    )
                    ex = spool.tile([P, N_CLASSES], f32, tag="ex", name="ex")
                    nc.scalar.activation(
                        out=ex[:], in_=ps[:],
                        func=mybir.ActivationFunctionType.Exp,
                        bias=negmax[:, 0:1],
                    )
                    ssum = spool.tile([P, 1], f32, tag="ssum", name="ssum")
                    nc.vector.tensor_reduce(
                        out=ssum[:], in_=ex[:], axis=mybir.AxisListType.X,
                        op=mybir.AluOpType.add,
                    )
                    rinv = spool.tile([P, 1], f32, tag="rinv", name="rinv")
                    nc.vector.reciprocal(out=rinv[:], in_=ssum[:])
                    nc.vector.tensor_scalar_mul(
                        outsb[:, m * N_CLASSES:(m + 1) * N_CLASSES], ex[:], rinv[:, 0:1]
                    )

            # ---- layer 1: three sweeps, each shipping an AG chunk.
            # Sweeps alternate psum tag sets A=(0..3)/B=(4..7); each sweep's
            # trailing gemms use its own (freed) set and are emitted after
            # the NEXT sweep's first j-pairs so evacuation never bubbles.
            TAGSETS = [[0, 1, 2, 3], [4, 5, 6, 7]]

            pending = None  # (sweep index, tagset) awaiting gemms + AG
            for si, (off, width, mts) in enumerate(SWEEPS1):
                get = get_sw(si)
                ts = TAGSETS[si % 2]
                ps = [
                    ppool.tile([P, width], f32, tag=f"sp{ts[f]}", name=f"s1_{si}_{f}")
                    for f in range(NF)
                ]
                for jp in range(NJP):
                    at = get(jp)
                    for f in range(NF):
                        nc.tensor.matmul(
                            out=ps[f][:],
                            lhsT=t_pair[0][jp][:, :, f * P:(f + 1) * P],
                            rhs=at,
                            start=(jp == 0),
                            stop=(jp == NJP - 1),
                            perf_mode=mybir.MatmulPerfMode.DoubleRow,
                        )
                    if jp == 3 and pending is not None:
                        po, pts = pending
                        poff, pw, pmts = SWEEPS1[po]
                        w1_gemm(poff, pw, pts)
                        for c, (cb, cn) in enumerate(AG_CHUNKS):
                            if cb in pmts:
                                gemm2_tiles(list(range(cb, cb + cn)), pts)
                                ag_chunk(c)
                        pending = None
                for f in range(NF):
                    nc.vector.tensor_copy(
                        out=yp[f // 2][:, f % 2, off:off + width],
                        in_=ps[f][:],
                    )
                pending = (si, ts)
            # last sweep's gemms + chunk (nothing left to hide them under)
            po, pts = pending
            poff, pw, pmts = SWEEPS1[po]
            w1_gemm(poff, pw, pts)
            for c, (cb, cn) in enumerate(AG_CHUNKS):
                if cb in pmts:
                    gemm2_tiles(list(range(cb, cb + cn)), pts)
                    ag_chunk(c)
            # x tiles are dead after the last sweep; refill them with ATc1
            # (same [P, 2, 512] shape) on the idle sync queue so layer-2's
            # pass 1 reads BOTH adjacency halves from SBUF -- streaming it
            # JIT under the resident atr reads cost ~24us of SBUF write/read
            # port contention (measured pass-1 at 92us vs 68us model).
            # Each refill DMA waits only on that tile's last sweep read, so
            # the stream rides right behind the final sweep's progress and
            # finishes during the pre-pass-1 AllGather wait; the chunk-gated
            # t2 reloads are emitted behind it and start at ~the same time.
            for jp in range(NJP):
                nc.sync.dma_start(out=t_pair[0][jp][:], in_=ATc1[jp, :, :, :])
            for c in range(len(AG_CHUNKS)):
                load_t2_chunk(c)

            # ---- layer 2: consume j-pairs in AG-chunk arrival order ----
            order2 = [
                5 * r + base // 2 + ii
                for (base, nt) in AG_CHUNKS
                for r in range(N_CORES)
                for ii in range(nt // 2)
            ]
            assert sorted(order2) == list(range(NJP))

            # Pass 1: cols 0:1024 -- streamed ATc1 (re-read) into banks 0-3
            # plus resident atr into banks 4-7; all 8 PSUM banks live.  This
            # puts 2/3 of layer-2 in the first pass so only the short 256-col
            # pass 2 (plus the softmax heads) remains after the last
            # AllGather chunk's j-pairs -- the post-wire tail drops from
            # ~55us to ~38us.
            psa = [ppool.tile([P, 512], f32, tag=f"sp{f}", name=f"s2a_{f}") for f in range(NF)]
            psb = [ppool.tile([P, 512], f32, tag=f"sp{4 + f}", name=f"s2b_{f}") for f in range(NF)]
            for idx, jp in enumerate(order2):
                at = t_pair[0][jp]
                ar = get_r2(jp)
                for f in range(NF):
                    nc.tensor.matmul(
                        out=psa[f][:],
                        lhsT=t_pair[1][jp][:, :, f * P:(f + 1) * P],
                        rhs=at[:, :, :],
                        start=(idx == 0),
                        stop=(idx == NJP - 1),
                        perf_mode=mybir.MatmulPerfMode.DoubleRow,
                    )
                    nc.tensor.matmul(
                        out=psb[f][:],
                        lhsT=t_pair[1][jp][:, :, f * P:(f + 1) * P],
                        rhs=ar[:, :, :],
                        start=(idx == 0),
                        stop=(idx == NJP - 1),
                        perf_mode=mybir.MatmulPerfMode.DoubleRow,
                    )
            for f in range(NF):
                nc.vector.tensor_scalar(
                    out=hp[1][f // 2][:, f % 2, 0:512],
                    in0=psa[f][:],
                    scalar1=bcols_sb[:, NF + f:NF + f + 1],
                    scalar2=0.0,
                    op0=mybir.AluOpType.add,
                    op1=mybir.AluOpType.max,
                )
                nc.vector.tensor_scalar(
                    out=hp[1][f // 2][:, f % 2, 512:1024],
                    in0=psb[f][:],
                    scalar1=bcols_sb[:, NF + f:NF + f + 1],
                    scalar2=0.0,
                    op0=mybir.AluOpType.add,
                    op1=mybir.AluOpType.max,
                )

            # Pass 2: cols 1024:1280, two column-sliced [P, 512] psum tiles
            # (banks 0-1); heads for m0..7 overlap it on banks 4-7.  Pass 2
            # runs entirely after pass 1 (whose tail needed the last AG
            # chunk), so every j-pair is present -- natural order is fine
            # and keeps the packed paired ATc2 loads.
            ps2 = [ppool.tile([P, 512], f32, tag=f"sp{t}", name=f"s2c_{t}") for t in range(NFP)]
            for jp in range(NJP):
                g, u = jp // 2, jp % 2
                if u == 0:
                    at2 = wpool.tile([P, 2, 2, 256], fp8, tag="atc2b", name=f"atc2b_{g}")
                    nc.scalar.dma_start(out=at2[:], in_=ATc2[g, :, :, :, :])
                    cur2 = at2
                for f in range(NF):
                    nc.tensor.matmul(
                        out=ps2[f // 2][:, (f % 2) * 256:(f % 2) * 256 + 256],
                        lhsT=t_pair[1][jp][:, :, f * P:(f + 1) * P],
                        rhs=cur2[:, u, :, :],
                        start=(jp == 0),
                        stop=(jp == NJP - 1),
                        perf_mode=mybir.MatmulPerfMode.DoubleRow,
                    )
                if jp == 3:
                    head_tiles([0, 1, 2, 3, 4, 5, 6, 7])
                if jp == 20:
                    # ship the finished heads' output columns during pass 2
                    # so only m8/m9's 32KB rides the final DMA
                    nc.sync.dma_start(
                        out=out[:, 0:8 * N_CLASSES], in_=outsb[:, 0:8 * N_CLASSES]
                    )
            for f in range(NF):
                nc.vector.tensor_scalar(
                    out=hp[1][f // 2][:, f % 2, 1024:1280],
                    in0=ps2[f // 2][:, (f % 2) * 256:(f % 2) * 256 + 256],
                    scalar1=bcols_sb[:, NF + f:NF + f + 1],
                    scalar2=0.0,
                    op0=mybir.AluOpType.add,
                    op1=mybir.AluOpType.max,
                )
            head_tiles([8, 9])

            nc.sync.dma_start(
                out=out[:, 8 * N_CLASSES:], in_=outsb[:, 8 * N_CLASSES:]
            )

    return nc


def build_inputs(x, edge_index, W1, b1, W2, b2, Wout, bout):
    """Host-side graph preprocessing + per-core shard construction."""
    x = np.asarray(x)
    ei = np.asarray(edge_index)
    n = N_NODES
    src = np.concatenate([ei[0], np.arange(n, dtype=np.int64)])
    dst = np.concatenate([ei[1], np.arange(n, dtype=np.int64)])
    deg = np.bincount(dst, minlength=n).astype(np.float32)
    dinv = 1.0 / np.sqrt(deg)
    normv = (dinv[src] * dinv[dst]).astype(np.float32)

    # dense Ahat^T, padded:  AhatT[src, dst] = norm  (duplicate edges sum)
    AhatT = np.zeros((NP, NP), dtype=np.float32)
    np.add.at(AhatT, (src, dst), normv)
    # DoubleRow pair-interleave: blocks[jp, p, q, :] = AhatT[jp*256+q*128+p, :]
    blocks = AhatT.astype(FP8).reshape(NJP, 2, P, NP).transpose(0, 2, 1, 3)

    xp = np.zeros((NP, F_IN), dtype=np.float32)
    xp[:n] = x
    # xpair[jp, p, q, c] = x[(2jp+q)*128+p, c]
    xpair = np.ascontiguousarray(
        xp.reshape(NJP, 2, P, F_IN).transpose(0, 2, 1, 3)
    ).astype(FP8)

    def wpairs(W):
        W = np.asarray(W, np.float32)
        # [t, p, q, n] = W[(2t+q)*128+p, n]
        return np.ascontiguousarray(
            W.reshape(NFP, 2, P, W.shape[1]).transpose(0, 2, 1, 3)
        ).astype(FP8)

    W1b = wpairs(W1)
    W2b = wpairs(W2)
    Woutb = wpairs(Wout)
    boutb = np.asarray(bout).reshape(1, N_CLASSES).astype(BF16)
    # biases as per-partition columns: bcols[:, l*NF + f] = b_l[f*128:(f+1)*128]
    bcols = np.stack(
        [np.asarray(b1).reshape(NF, P), np.asarray(b2).reshape(NF, P)], 0
    ).reshape(2 * NF, P).T.astype(np.float32)
    bcols = np.ascontiguousarray(bcols)

    in_maps = []
    for k in range(N_CORES):
        # blkp[g, p, u, q, :] = AhatT row (2g+u)*256+q*128+p, this core's cols
        blkp = blocks[:, :, :, k * R:(k + 1) * R].reshape(
            NJP // 2, 2, P, 2, R
        ).transpose(0, 2, 1, 3, 4)
        ATr_ = np.ascontiguousarray(blkp[..., 512:1024])
        ATc2_ = np.ascontiguousarray(blkp[..., 1024:1280])
        ATc1a_ = np.ascontiguousarray(blkp[..., 0:256])
        ATc1b_ = np.ascontiguousarray(blkp[..., 256:512])
        blk = blocks[:, :, :, k * R:(k + 1) * R]
        ATc1_ = np.ascontiguousarray(blk[..., 0:512])
        in_maps.append({
            "xpair": xpair,
            "ATr": ATr_,
            "ATc1": ATc1_,
            "ATc1a": ATc1a_,
            "ATc1b": ATc1b_,
            "ATc2": ATc2_,
            "W1p": W1b,
            "W2p": W2b,
            "Woutp": Woutb,
            "bcols": bcols,
            "bout": boutb,
        })
    return in_maps


_CACHED = {}


def _get_program():
    if "nc" not in _CACHED:
        nc = bass.Bass(num_devices=N_CORES)
        build_gcn(nc)
        split_drain_waits(nc)
        _CACHED["nc"] = nc
    return _CACHED["nc"]


def kernel(x, edge_index, W1, b1, W2, b2, Wout, bout, trace=False):
    install_ntff_hook()
    nc = _get_program()
    in_maps = build_inputs(x, edge_index, W1, b1, W2, b2, Wout, bout)
    res = run_bass_kernel_spmd(
        nc, in_maps, core_ids=list(range(N_CORES)), trace=trace
    )
    # out[p, m*16+c] -> rows m*128+p
    outs = []
    for k in range(N_CORES):
        o = res.results[k]["out"]
        outs.append(o.reshape(P, NM, N_CLASSES).transpose(1, 0, 2).reshape(R, N_CLASSES))
    out = np.concatenate(outs, 0)
    kernel.last_exec_time_ns = res.exec_time_ns
    kernel.last_results = res
    return out[:N_NODES].astype(np.float32)


kernel.last_exec_time_ns = None
kernel.last_results = None
